# revision 22
# baseline (speedup 1.0000x reference)
"""NT-Xent / SimCLR contrastive loss on 8 Trainium2 NeuronCores.

Math (matches the jax reference):
    z = l2_normalize(concat([emb_i, emb_j]))          # [2B, D] unit rows
    sim = z @ z.T                                     # cosine similarities
    denom_r = sum_{j != r} exp(sim_rj / T)
    pos_r   = z_r . z_{(r+B) mod 2B}                  # the positive pair
    loss = mean_r( log(denom_r) - pos_r / T )

v5 — symmetric sharding + fp8 DoubleRow matmuls, host staging:
  sim is symmetric, so only the ~33.5M unique entries are exp'd (the
  exp on the ACT engine is the hard bottleneck: 1 elem/lane/cycle).
  The 8192 rows form 16 blocks of 512; core c owns row-blocks c and
  c+8 and computes blocks (c, c+l mod 16) for l=0..8 and
  (c+8, c+8+l mod 16) for l=0..7.  Every unordered block pair is
  covered exactly once (offsets 1..7 from each row-block, the 8 wrap
  pairs {c, c+8} internal to core c, 16 diagonals) — 17 blocks per
  core, balanced and SPMD-uniform.

  The host prepares the sharded operand layout: normalized rows,
  pre-transposed [d, row] panels, fp8e4 cast (bit-identical to the
  device DVE cast, verified RTN).  The device does the compute:
    - 68 fp8 DoubleRow matmuls (K=256 in one pass, 157 TF/s)
    - ACT exp(2*sim) straight out of PSUM, fused row accumulation,
      bf16 exp values to SBUF
    - 60 ones-matmuls on the PE for the 15 off-diagonal column sums
      (the transpose-side contributions), DVE drains
    - positives as exact fp32 row dots of the normalized rows
  Host assembles denom[8192] from row/col partials, subtracts the e^2
  self term, and takes mean(log(denom) - 2*pos) in float64.
"""

import numpy as np
from contextlib import ExitStack

import ml_dtypes
import concourse.bass as bass
import concourse.tile as tile
from concourse import bacc, mybir
from concourse._compat import with_exitstack
from concourse.bass_utils import run_bass_kernel_spmd

B = 4096
D = 256
R = 2 * B
N_CORES = 8
NBLK = 16            # global 512-row blocks
BLK = 512
INV_T = 2.0
E2 = float(np.exp(2.0))

F32 = mybir.dt.float32
BF16 = mybir.dt.bfloat16
FP8 = mybir.dt.float8e4
I32 = mybir.dt.int32
DR = mybir.MatmulPerfMode.DoubleRow
# Schraudolph exp on the DVE for a few groups (offloads the ACT
# bottleneck): exp(2*s) ~= bitcast_f32(int32(s*SCH_A + SCH_B)).
# SCH_B tuned on-device so the mean approximation ratio is 1.0.
SCH_A = 2.0 * (1 << 23) / np.log(2.0)
SCH_B = float(127 * (1 << 23) - 322200)
DVE_OFF = {(0, 2), (0, 6), (1, 1), (1, 5), (2, 2), (2, 3)}

# local col-block lists per exp group; row-block c tiles (m 0..3) use local
# cols 0..8 (diag, +1..+7, wrap), row-block c+8 tiles (m 4..7) use 8..15.
R0_GROUPS = [[0, 1, 2], [3, 4, 5], [6, 7, 8]]
R1_GROUPS = [[8, 9, 10], [11, 12, 13], [14, 15]]
# off-diagonal (row-tile-base, local col block) needing column sums
CS_BLOCKS = [(0, l) for l in range(1, 9)] + [(4, l) for l in range(9, 16)]
NCS = len(CS_BLOCKS)  # 15

# DMA block order: interleave the two half-panels so G0 unblocks first
B_SEQ = [b for g in range(8) for b in (g, g + 8)]
# local row-tile index backing matmul row m (tiles 0..3 = block c rows,
# zT cols 4096.. hold block c+8 = local block 8)
LIDX = [0, 1, 2, 3, 32, 33, 34, 35]


@with_exitstack
def _loss_kernel(ctx: ExitStack, tc: "tile.TileContext", denacc_ap: bass.AP,
                 cs_ap: bass.AP, pos_ap: bass.AP, zt_ap: bass.AP,
                 zrow_ap: bass.AP):
    nc = tc.nc
    mult = mybir.AluOpType.mult
    add = mybir.AluOpType.add
    Exp = mybir.ActivationFunctionType.Exp

    xpool = ctx.enter_context(tc.tile_pool(name="x", bufs=1))
    ipool = ctx.enter_context(tc.tile_pool(name="i32", bufs=2))
    jpool = ctx.enter_context(tc.tile_pool(name="junk", bufs=2))
    ztpool = ctx.enter_context(tc.tile_pool(name="zt", bufs=1))
    epool = ctx.enter_context(tc.tile_pool(name="esc", bufs=1))
    cpool = ctx.enter_context(tc.tile_pool(name="const", bufs=1))
    opool = ctx.enter_context(tc.tile_pool(name="outs", bufs=1))

    mpsum = ctx.enter_context(tc.tile_pool(name="mm", bufs=2, space="PSUM"))

    ones = cpool.tile([128, 1], BF16, tag="ones")
    nc.vector.memset(ones[:], 1.0)
    onesw = cpool.tile([128, 3 * BLK], F32, tag="onesw")
    nc.vector.memset(onesw[:], 1.0)

    zT = ztpool.tile([128, 2, R], FP8, tag="zt")
    # The gpsimd SWDGE issue costs ~770ns per dma_start, serialized; put
    # the six blocks the first matmul groups need on the idle HWDGE
    # queues (in need order), and fetch the rest as two span DMAs.
    nc.sync.dma_start(zT[:, :, 0:BLK * 3], zt_ap[:, :, 0:BLK * 3])
    nc.scalar.dma_start(zT[:, :, BLK * 8:BLK * 11],
                        zt_ap[:, :, BLK * 8:BLK * 11])
    for b in (3, 11, 4, 12, 5, 13, 6, 14, 7, 15):
        nc.gpsimd.dma_start(zT[:, :, BLK * b:BLK * (b + 1)],
                            zt_ap[:, :, BLK * b:BLK * (b + 1)])
    x = xpool.tile([128, 8, D], F32, tag="x")
    nc.sync.dma_start(
        x[:], zrow_ap[:].rearrange("(t p) d -> p t d", p=128))

    esc = epool.tile([128, 8, 9 * BLK], BF16, tag="esc")
    denacc = opool.tile([128, 26], F32, tag="denacc")
    pos = opool.tile([128, 4], F32, tag="pos")
    csb = opool.tile([1, NCS * BLK], F32, tag="csb")

    cpsum = ctx.enter_context(tc.tile_pool(name="cs", bufs=2, space="PSUM"))

    def emit_mm_group(gi, mlist=None, gmap=None, dcol=None):
        for m in (mlist if mlist is not None else range(8)):
            groups = gmap[m] if gmap else (
                R0_GROUPS[gi] if m < 4 else R1_GROUPS[gi])
            width = BLK * len(groups)
            ptf = mpsum.tile([128, 3 * BLK], F32, tag="mm",
                             name=f"pt{gi}_{m}_{len(groups)}")
            pt = ptf[:, :width]
            for i, t in enumerate(groups):
                nc.tensor.matmul(
                    pt[:, BLK * i:BLK * (i + 1)],
                    lhsT=zT[:, :, 128 * LIDX[m]:128 * (LIDX[m] + 1)],
                    rhs=zT[:, :, BLK * t:BLK * (t + 1)],
                    start=True, stop=True, perf_mode=DR,
                )
            slot = groups[0] if m < 4 else groups[0] - 8
            k = dcol[m] if dcol else gi * 8 + m
            eslot = esc[:, m, BLK * slot:BLK * slot + width]
            if (gi, m) in DVE_OFF and dcol is None:
                yi = ipool.tile([128, 3 * BLK], I32, tag="yi",
                                name=f"yi{gi}_{m}")
                nc.vector.tensor_scalar(
                    out=yi[:, :width], in0=pt[:], scalar1=SCH_A,
                    scalar2=SCH_B, op0=mult, op1=add,
                )
                nc.vector.scalar_tensor_tensor(
                    out=eslot, in0=yi[:, :width].bitcast(F32), scalar=1.0,
                    in1=onesw[:, :width], op0=mult, op1=mult,
                    accum_out=denacc[:, k:k + 1],
                )
            else:
                nc.scalar.activation(
                    eslot, pt[:], Exp, scale=INV_T,
                    accum_out=denacc[:, k:k + 1],
                )
        if dcol is None:
            nc.sync.dma_start(denacc_ap[:, gi * 8:gi * 8 + 8],
                              denacc[:, gi * 8:gi * 8 + 8])

    def emit_cs_group(gi):
        # column sums (DoubleRow: two m-tiles per matmul) of the
        # off-diagonal blocks whose exps were produced by group gi
        for bi, (mb, l) in enumerate(CS_BLOCKS):
            lset = R0_GROUPS[gi] if mb == 0 else R1_GROUPS[gi]
            if l not in lset or l == 0 or l == 8 and mb == 4:
                continue
            slot = l if mb == 0 else l - 8
            cs = cpsum.tile([1, BLK], F32, tag="cs", name=f"cs{bi}")
            for mm in range(4):
                nc.tensor.matmul(
                    cs[:], lhsT=ones[:, 0:1],
                    rhs=esc[:, mb + mm, BLK * slot:BLK * (slot + 1)],
                    start=(mm == 0), stop=(mm == 3),
                )
            nc.vector.tensor_copy(csb[0:1, BLK * bi:BLK * (bi + 1)], cs[:])
            nc.sync.dma_start(cs_ap[0:1, BLK * bi:BLK * (bi + 1)],
                              csb[0:1, BLK * bi:BLK * (bi + 1)])

    # ---- main loop: matmuls + fused exp/rowsum, colsums interleaved -----
    emit_mm_group(0)
    emit_mm_group(1)
    emit_cs_group(0)

    # positives: exact fp32 dots of normalized rows
    for m in range(4):
        junk = jpool.tile([128, D], F32, tag="junk", name=f"pp{m}")
        nc.vector.scalar_tensor_tensor(
            out=junk[:], in0=x[:, m, :], scalar=1.0,
            in1=x[:, 4 + m, :], op0=mult, op1=mult,
            accum_out=pos[:, m:m + 1],
        )

    emit_mm_group(2)
    emit_cs_group(1)
    emit_cs_group(2)

    nc.sync.dma_start(pos_ap[:], pos[:])


_CACHE = {}


def _get_compiled():
    if "nc" not in _CACHE:
        nc = bacc.Bacc("TRN2", target_bir_lowering=False, debug=False)
        zt_in = nc.dram_tensor("zt8", [128, 2, R], FP8, kind="ExternalInput")
        zrow_in = nc.dram_tensor("zrow", [1024, D], F32, kind="ExternalInput")
        den_out = nc.dram_tensor("denacc", [128, 26], F32, kind="ExternalOutput")
        cs_out = nc.dram_tensor("colsum", [1, NCS * BLK], F32, kind="ExternalOutput")
        pos_out = nc.dram_tensor("pos", [128, 4], F32, kind="ExternalOutput")
        with tile.TileContext(nc) as tc:
            _loss_kernel(tc, den_out.ap(), cs_out.ap(), pos_out.ap(),
                         zt_in.ap(), zrow_in.ap())
        nc.compile()
        _CACHE["nc"] = nc
    return _CACHE["nc"]


def make_in_maps(emb_i: np.ndarray, emb_j: np.ndarray):
    reps = np.concatenate(
        [np.asarray(emb_i, dtype=np.float32), np.asarray(emb_j, dtype=np.float32)],
        axis=0,
    )
    n = np.sqrt(np.sum(reps.astype(np.float64) ** 2, axis=1, keepdims=True))
    z = (reps / n).astype(np.float32)
    in_maps = []
    for c in range(N_CORES):
        zr = np.roll(z, -c * BLK, axis=0)
        # [128, 2, 8192] fp8: zt[p, k, col] = z[col, 128k + p]
        zt8 = np.ascontiguousarray(
            zr.reshape(R, 2, 128).transpose(2, 1, 0)
        ).astype(ml_dtypes.float8_e4m3)
        zrow = np.ascontiguousarray(
            np.concatenate([zr[0:BLK], zr[8 * BLK:9 * BLK]], axis=0))
        in_maps.append({"zt8": zt8, "zrow": zrow})
    return in_maps


def run_spmd(emb_i, emb_j, **kwargs):
    nc = _get_compiled()
    in_maps = make_in_maps(emb_i, emb_j)
    return run_bass_kernel_spmd(nc, in_maps, core_ids=list(range(N_CORES)), **kwargs)


def assemble(results) -> np.ndarray:
    denom = np.zeros(R, dtype=np.float64)
    pos2 = np.zeros(R, dtype=np.float64)
    for c in range(N_CORES):
        r = results[c]
        da = r["denacc"].astype(np.float64)        # [128, 24]
        cs = r["colsum"].reshape(NCS, BLK).astype(np.float64)
        pr = r["pos"].astype(np.float64)           # [128, 4]
        for m in range(8):
            s = da[:, m] + da[:, 8 + m] + da[:, 16 + m]
            if m == 0:
                s = s + da[:, 24]
            elif m == 4:
                s = s + da[:, 25]
            blk = c if m < 4 else c + 8
            g0 = blk * BLK + (m % 4) * 128
            denom[g0:g0 + 128] += s
        for bi, (mb, l) in enumerate(CS_BLOCKS):
            j = (c + l) % NBLK
            denom[j * BLK:(j + 1) * BLK] += cs[bi]
        for m in range(4):
            p2 = 2.0 * pr[:, m]
            g = BLK * c + m * 128 + np.arange(128)
            pos2[g] = p2
            pos2[(g + B) % R] = p2
    denom -= E2
    loss = float(np.mean(np.log(denom) - pos2))
    return np.array(loss, dtype=np.float32)


def kernel(emb_i: np.ndarray, emb_j: np.ndarray) -> np.ndarray:
    res = run_spmd(emb_i, emb_j)
    return assemble(res.results)


# revision 23
# speedup vs baseline: 1.1289x; 1.1289x over previous
"""NT-Xent / SimCLR contrastive loss on 8 Trainium2 NeuronCores.

Math (matches the jax reference):
    z = l2_normalize(concat([emb_i, emb_j]))          # [2B, D] unit rows
    sim = z @ z.T                                     # cosine similarities
    denom_r = sum_{j != r} exp(sim_rj / T)
    pos_r   = z_r . z_{(r+B) mod 2B}                  # the positive pair
    loss = mean_r( log(denom_r) - pos_r / T )

v5 — symmetric sharding + fp8 DoubleRow matmuls, host staging:
  sim is symmetric, so only the ~33.5M unique entries are exp'd (the
  exp on the ACT engine is the hard bottleneck: 1 elem/lane/cycle).
  The 8192 rows form 16 blocks of 512; core c owns row-blocks c and
  c+8 and computes blocks (c, c+l mod 16) for l=0..8 and
  (c+8, c+8+l mod 16) for l=0..7.  Every unordered block pair is
  covered exactly once (offsets 1..7 from each row-block, the 8 wrap
  pairs {c, c+8} internal to core c, 16 diagonals) — 17 blocks per
  core, balanced and SPMD-uniform.

  The host prepares the sharded operand layout: normalized rows,
  pre-transposed [d, row] panels, fp8e4 cast (bit-identical to the
  device DVE cast, verified RTN).  The device does the compute:
    - 68 fp8 DoubleRow matmuls (K=256 in one pass, 157 TF/s)
    - ACT exp(2*sim) straight out of PSUM, fused row accumulation,
      bf16 exp values to SBUF
    - 60 ones-matmuls on the PE for the 15 off-diagonal column sums
      (the transpose-side contributions), DVE drains
    - positives as exact fp32 row dots of the normalized rows
  Host assembles denom[8192] from row/col partials, subtracts the e^2
  self term, and takes mean(log(denom) - 2*pos) in float64.
"""

import numpy as np
from contextlib import ExitStack

import ml_dtypes
import concourse.bass as bass
import concourse.tile as tile
from concourse import bacc, mybir
from concourse._compat import with_exitstack
from concourse.bass_utils import run_bass_kernel_spmd

B = 4096
D = 256
R = 2 * B
N_CORES = 8
NBLK = 16            # global 512-row blocks
BLK = 512
INV_T = 2.0
E2 = float(np.exp(2.0))

F32 = mybir.dt.float32
BF16 = mybir.dt.bfloat16
FP8 = mybir.dt.float8e4
I32 = mybir.dt.int32
DR = mybir.MatmulPerfMode.DoubleRow
# Schraudolph exp on the DVE for a few groups (offloads the ACT
# bottleneck): exp(2*s) ~= bitcast_f32(int32(s*SCH_A + SCH_B)).
# SCH_B tuned on-device so the mean approximation ratio is 1.0.
SCH_A = 2.0 * (1 << 23) / np.log(2.0)
SCH_B = float(127 * (1 << 23) - 322200)
DVE_OFF = {(0, 5), (1, 2), (1, 6), (2, 1)}

# local col-block lists per exp group; row-block c tiles (m 0..3) use local
# cols 0..8 (diag, +1..+7, wrap), row-block c+8 tiles (m 4..7) use 8..15.
R0_GROUPS = [[0, 1, 2], [3, 4, 5], [6, 7, 8]]
R1_GROUPS = [[8, 9, 10], [11, 12, 13], [14, 15]]
# off-diagonal (row-tile-base, local col block) needing column sums
CS_BLOCKS = [(0, l) for l in range(1, 9)] + [(4, l) for l in range(9, 16)]
NCS = len(CS_BLOCKS)  # 15

# DMA block order: interleave the two half-panels so G0 unblocks first
B_SEQ = [b for g in range(8) for b in (g, g + 8)]
# local row-tile index backing matmul row m (tiles 0..3 = block c rows,
# zT cols 4096.. hold block c+8 = local block 8)
LIDX = [0, 1, 2, 3, 32, 33, 34, 35]


@with_exitstack
def _loss_kernel(ctx: ExitStack, tc: "tile.TileContext", denacc_ap: bass.AP,
                 cs_ap: bass.AP, pos_ap: bass.AP, zt_ap: bass.AP,
                 zrow_ap: bass.AP):
    nc = tc.nc
    mult = mybir.AluOpType.mult
    add = mybir.AluOpType.add
    Exp = mybir.ActivationFunctionType.Exp

    xpool = ctx.enter_context(tc.tile_pool(name="x", bufs=1))
    ipool = ctx.enter_context(tc.tile_pool(name="i32", bufs=2))
    jpool = ctx.enter_context(tc.tile_pool(name="junk", bufs=2))
    ztpool = ctx.enter_context(tc.tile_pool(name="zt", bufs=1))
    epool = ctx.enter_context(tc.tile_pool(name="esc", bufs=1))
    cpool = ctx.enter_context(tc.tile_pool(name="const", bufs=1))
    opool = ctx.enter_context(tc.tile_pool(name="outs", bufs=1))

    mpsum = ctx.enter_context(tc.tile_pool(name="mm", bufs=2, space="PSUM"))

    ones = cpool.tile([128, 1], BF16, tag="ones")
    nc.vector.memset(ones[:], 1.0)
    onesw = cpool.tile([128, 3 * BLK], F32, tag="onesw")
    nc.vector.memset(onesw[:], 1.0)

    zT = ztpool.tile([128, 2, R], FP8, tag="zt")
    # The gpsimd SWDGE issue costs ~770ns per dma_start, serialized; put
    # the six blocks the first matmul groups need on the idle HWDGE
    # queues (in need order), and fetch the rest as two span DMAs.
    nc.sync.dma_start(zT[:, :, 0:BLK * 3], zt_ap[:, :, 0:BLK * 3])
    nc.scalar.dma_start(zT[:, :, BLK * 8:BLK * 11],
                        zt_ap[:, :, BLK * 8:BLK * 11])
    for b in (3, 11, 4, 12, 5, 13, 6, 14, 7, 15):
        nc.gpsimd.dma_start(zT[:, :, BLK * b:BLK * (b + 1)],
                            zt_ap[:, :, BLK * b:BLK * (b + 1)])
    x = xpool.tile([128, 8, D], F32, tag="x")
    nc.sync.dma_start(
        x[:], zrow_ap[:].rearrange("(t p) d -> p t d", p=128))

    esc = epool.tile([128, 8, 9 * BLK], BF16, tag="esc")
    denacc = opool.tile([128, 26], F32, tag="denacc")
    pos = opool.tile([128, 4], F32, tag="pos")
    csb = opool.tile([1, NCS * BLK], F32, tag="csb")

    cpsum = ctx.enter_context(tc.tile_pool(name="cs", bufs=2, space="PSUM"))

    def emit_mm_group(gi, mlist=None, gmap=None, dcol=None):
        for m in (mlist if mlist is not None else range(8)):
            groups = gmap[m] if gmap else (
                R0_GROUPS[gi] if m < 4 else R1_GROUPS[gi])
            width = BLK * len(groups)
            ptf = mpsum.tile([128, 3 * BLK], F32, tag="mm",
                             name=f"pt{gi}_{m}_{len(groups)}")
            pt = ptf[:, :width]
            for i, t in enumerate(groups):
                nc.tensor.matmul(
                    pt[:, BLK * i:BLK * (i + 1)],
                    lhsT=zT[:, :, 128 * LIDX[m]:128 * (LIDX[m] + 1)],
                    rhs=zT[:, :, BLK * t:BLK * (t + 1)],
                    start=True, stop=True, perf_mode=DR,
                )
            slot = groups[0] if m < 4 else groups[0] - 8
            k = dcol[m] if dcol else gi * 8 + m
            eslot = esc[:, m, BLK * slot:BLK * slot + width]
            if (gi, m) in DVE_OFF and dcol is None:
                yi = ipool.tile([128, 3 * BLK], I32, tag="yi",
                                name=f"yi{gi}_{m}")
                nc.vector.tensor_scalar(
                    out=yi[:, :width], in0=pt[:], scalar1=SCH_A,
                    scalar2=SCH_B, op0=mult, op1=add,
                )
                nc.vector.scalar_tensor_tensor(
                    out=eslot, in0=yi[:, :width].bitcast(F32), scalar=1.0,
                    in1=onesw[:, :width], op0=mult, op1=mult,
                    accum_out=denacc[:, k:k + 1],
                )
            else:
                nc.scalar.activation(
                    eslot, pt[:], Exp, scale=INV_T,
                    accum_out=denacc[:, k:k + 1],
                )
        if dcol is None:
            nc.sync.dma_start(denacc_ap[:, gi * 8:gi * 8 + 8],
                              denacc[:, gi * 8:gi * 8 + 8])

    def emit_cs_group(gi):
        # column sums (DoubleRow: two m-tiles per matmul) of the
        # off-diagonal blocks whose exps were produced by group gi
        for bi, (mb, l) in enumerate(CS_BLOCKS):
            lset = R0_GROUPS[gi] if mb == 0 else R1_GROUPS[gi]
            if l not in lset or l == 0 or l == 8 and mb == 4:
                continue
            slot = l if mb == 0 else l - 8
            cs = cpsum.tile([1, BLK], F32, tag="cs", name=f"cs{bi}")
            for mm in range(4):
                nc.tensor.matmul(
                    cs[:], lhsT=ones[:, 0:1],
                    rhs=esc[:, mb + mm, BLK * slot:BLK * (slot + 1)],
                    start=(mm == 0), stop=(mm == 3),
                )
            nc.vector.tensor_copy(csb[0:1, BLK * bi:BLK * (bi + 1)], cs[:])
            nc.sync.dma_start(cs_ap[0:1, BLK * bi:BLK * (bi + 1)],
                              csb[0:1, BLK * bi:BLK * (bi + 1)])

    # ---- main loop: matmuls + fused exp/rowsum, colsums interleaved -----
    emit_mm_group(0)
    emit_mm_group(1)
    emit_cs_group(0)

    # positives: exact fp32 dots of normalized rows
    for m in range(4):
        junk = jpool.tile([128, D], F32, tag="junk", name=f"pp{m}")
        nc.vector.scalar_tensor_tensor(
            out=junk[:], in0=x[:, m, :], scalar=1.0,
            in1=x[:, 4 + m, :], op0=mult, op1=mult,
            accum_out=pos[:, m:m + 1],
        )

    emit_mm_group(2)
    emit_cs_group(1)
    emit_cs_group(2)

    nc.sync.dma_start(pos_ap[:], pos[:])


_CACHE = {}


def _get_compiled():
    if "nc" not in _CACHE:
        nc = bacc.Bacc("TRN2", target_bir_lowering=False, debug=False)
        zt_in = nc.dram_tensor("zt8", [128, 2, R], FP8, kind="ExternalInput")
        zrow_in = nc.dram_tensor("zrow", [1024, D], F32, kind="ExternalInput")
        den_out = nc.dram_tensor("denacc", [128, 26], F32, kind="ExternalOutput")
        cs_out = nc.dram_tensor("colsum", [1, NCS * BLK], F32, kind="ExternalOutput")
        pos_out = nc.dram_tensor("pos", [128, 4], F32, kind="ExternalOutput")
        with tile.TileContext(nc) as tc:
            _loss_kernel(tc, den_out.ap(), cs_out.ap(), pos_out.ap(),
                         zt_in.ap(), zrow_in.ap())
        nc.compile()
        _CACHE["nc"] = nc
    return _CACHE["nc"]


def make_in_maps(emb_i: np.ndarray, emb_j: np.ndarray):
    reps = np.concatenate(
        [np.asarray(emb_i, dtype=np.float32), np.asarray(emb_j, dtype=np.float32)],
        axis=0,
    )
    n = np.sqrt(np.sum(reps.astype(np.float64) ** 2, axis=1, keepdims=True))
    z = (reps / n).astype(np.float32)
    in_maps = []
    for c in range(N_CORES):
        zr = np.roll(z, -c * BLK, axis=0)
        # [128, 2, 8192] fp8: zt[p, k, col] = z[col, 128k + p]
        zt8 = np.ascontiguousarray(
            zr.reshape(R, 2, 128).transpose(2, 1, 0)
        ).astype(ml_dtypes.float8_e4m3)
        zrow = np.ascontiguousarray(
            np.concatenate([zr[0:BLK], zr[8 * BLK:9 * BLK]], axis=0))
        in_maps.append({"zt8": zt8, "zrow": zrow})
    return in_maps


def run_spmd(emb_i, emb_j, **kwargs):
    nc = _get_compiled()
    in_maps = make_in_maps(emb_i, emb_j)
    return run_bass_kernel_spmd(nc, in_maps, core_ids=list(range(N_CORES)), **kwargs)


def assemble(results) -> np.ndarray:
    denom = np.zeros(R, dtype=np.float64)
    pos2 = np.zeros(R, dtype=np.float64)
    for c in range(N_CORES):
        r = results[c]
        da = r["denacc"].astype(np.float64)        # [128, 24]
        cs = r["colsum"].reshape(NCS, BLK).astype(np.float64)
        pr = r["pos"].astype(np.float64)           # [128, 4]
        for m in range(8):
            s = da[:, m] + da[:, 8 + m] + da[:, 16 + m]
            if m == 0:
                s = s + da[:, 24]
            elif m == 4:
                s = s + da[:, 25]
            blk = c if m < 4 else c + 8
            g0 = blk * BLK + (m % 4) * 128
            denom[g0:g0 + 128] += s
        for bi, (mb, l) in enumerate(CS_BLOCKS):
            j = (c + l) % NBLK
            denom[j * BLK:(j + 1) * BLK] += cs[bi]
        for m in range(4):
            p2 = 2.0 * pr[:, m]
            g = BLK * c + m * 128 + np.arange(128)
            pos2[g] = p2
            pos2[(g + B) % R] = p2
    denom -= E2
    loss = float(np.mean(np.log(denom) - pos2))
    return np.array(loss, dtype=np.float32)


def kernel(emb_i: np.ndarray, emb_j: np.ndarray) -> np.ndarray:
    res = run_spmd(emb_i, emb_j)
    return assemble(res.results)


# revision 24
# speedup vs baseline: 1.1371x; 1.0072x over previous
"""NT-Xent / SimCLR contrastive loss on 8 Trainium2 NeuronCores.

Math (matches the jax reference):
    z = l2_normalize(concat([emb_i, emb_j]))          # [2B, D] unit rows
    sim = z @ z.T                                     # cosine similarities
    denom_r = sum_{j != r} exp(sim_rj / T)
    pos_r   = z_r . z_{(r+B) mod 2B}                  # the positive pair
    loss = mean_r( log(denom_r) - pos_r / T )

v5 — symmetric sharding + fp8 DoubleRow matmuls, host staging:
  sim is symmetric, so only the ~33.5M unique entries are exp'd (the
  exp on the ACT engine is the hard bottleneck: 1 elem/lane/cycle).
  The 8192 rows form 16 blocks of 512; core c owns row-blocks c and
  c+8 and computes blocks (c, c+l mod 16) for l=0..8 and
  (c+8, c+8+l mod 16) for l=0..7.  Every unordered block pair is
  covered exactly once (offsets 1..7 from each row-block, the 8 wrap
  pairs {c, c+8} internal to core c, 16 diagonals) — 17 blocks per
  core, balanced and SPMD-uniform.

  The host prepares the sharded operand layout: normalized rows,
  pre-transposed [d, row] panels, fp8e4 cast (bit-identical to the
  device DVE cast, verified RTN).  The device does the compute:
    - 68 fp8 DoubleRow matmuls (K=256 in one pass, 157 TF/s)
    - ACT exp(2*sim) straight out of PSUM, fused row accumulation,
      bf16 exp values to SBUF
    - 60 ones-matmuls on the PE for the 15 off-diagonal column sums
      (the transpose-side contributions), DVE drains
    - positives as exact fp32 row dots of the normalized rows
  Host assembles denom[8192] from row/col partials, subtracts the e^2
  self term, and takes mean(log(denom) - 2*pos) in float64.
"""

import numpy as np
from contextlib import ExitStack

import ml_dtypes
import concourse.bass as bass
import concourse.tile as tile
from concourse import bacc, mybir
from concourse._compat import with_exitstack
from concourse.bass_utils import run_bass_kernel_spmd

B = 4096
D = 256
R = 2 * B
N_CORES = 8
NBLK = 16            # global 512-row blocks
BLK = 512
INV_T = 2.0
E2 = float(np.exp(2.0))

F32 = mybir.dt.float32
BF16 = mybir.dt.bfloat16
FP8 = mybir.dt.float8e4
I32 = mybir.dt.int32
DR = mybir.MatmulPerfMode.DoubleRow
# Schraudolph exp on the DVE for a few groups (offloads the ACT
# bottleneck): exp(2*s) ~= bitcast_f32(int32(s*SCH_A + SCH_B)).
# SCH_B tuned on-device so the mean approximation ratio is 1.0.
SCH_A = 2.0 * (1 << 23) / np.log(2.0)
SCH_B = float(127 * (1 << 23) - 322200)
DVE_OFF = {(0, 5), (1, 2), (1, 6), (2, 1), (2, 5)}

# local col-block lists per exp group; row-block c tiles (m 0..3) use local
# cols 0..8 (diag, +1..+7, wrap), row-block c+8 tiles (m 4..7) use 8..15.
R0_GROUPS = [[0, 1, 2], [3, 4, 5], [6, 7, 8]]
R1_GROUPS = [[8, 9, 10], [11, 12, 13], [14, 15]]
# off-diagonal (row-tile-base, local col block) needing column sums
CS_BLOCKS = [(0, l) for l in range(1, 9)] + [(4, l) for l in range(9, 16)]
NCS = len(CS_BLOCKS)  # 15

# DMA block order: interleave the two half-panels so G0 unblocks first
B_SEQ = [b for g in range(8) for b in (g, g + 8)]
# local row-tile index backing matmul row m (tiles 0..3 = block c rows,
# zT cols 4096.. hold block c+8 = local block 8)
LIDX = [0, 1, 2, 3, 32, 33, 34, 35]


@with_exitstack
def _loss_kernel(ctx: ExitStack, tc: "tile.TileContext", denacc_ap: bass.AP,
                 cs_ap: bass.AP, pos_ap: bass.AP, zt_ap: bass.AP,
                 zrow_ap: bass.AP):
    nc = tc.nc
    mult = mybir.AluOpType.mult
    add = mybir.AluOpType.add
    Exp = mybir.ActivationFunctionType.Exp

    xpool = ctx.enter_context(tc.tile_pool(name="x", bufs=1))
    ipool = ctx.enter_context(tc.tile_pool(name="i32", bufs=2))
    jpool = ctx.enter_context(tc.tile_pool(name="junk", bufs=2))
    ztpool = ctx.enter_context(tc.tile_pool(name="zt", bufs=1))
    epool = ctx.enter_context(tc.tile_pool(name="esc", bufs=1))
    cpool = ctx.enter_context(tc.tile_pool(name="const", bufs=1))
    opool = ctx.enter_context(tc.tile_pool(name="outs", bufs=1))

    mpsum = ctx.enter_context(tc.tile_pool(name="mm", bufs=2, space="PSUM"))

    ones = cpool.tile([128, 1], BF16, tag="ones")
    nc.vector.memset(ones[:], 1.0)
    onesw = cpool.tile([128, 3 * BLK], F32, tag="onesw")
    nc.vector.memset(onesw[:], 1.0)

    zT = ztpool.tile([128, 2, R], FP8, tag="zt")
    # The gpsimd SWDGE issue costs ~770ns per dma_start, serialized; put
    # the six blocks the first matmul groups need on the idle HWDGE
    # queues (in need order), and fetch the rest as two span DMAs.
    nc.sync.dma_start(zT[:, :, 0:BLK * 3], zt_ap[:, :, 0:BLK * 3])
    nc.scalar.dma_start(zT[:, :, BLK * 8:BLK * 11],
                        zt_ap[:, :, BLK * 8:BLK * 11])
    for b in (3, 11, 4, 12, 5, 13, 6, 14, 7, 15):
        nc.gpsimd.dma_start(zT[:, :, BLK * b:BLK * (b + 1)],
                            zt_ap[:, :, BLK * b:BLK * (b + 1)])
    x = xpool.tile([128, 8, D], F32, tag="x")
    nc.sync.dma_start(
        x[:], zrow_ap[:].rearrange("(t p) d -> p t d", p=128))

    esc = epool.tile([128, 8, 9 * BLK], BF16, tag="esc")
    denacc = opool.tile([128, 26], F32, tag="denacc")
    pos = opool.tile([128, 4], F32, tag="pos")
    csb = opool.tile([1, NCS * BLK], F32, tag="csb")

    cpsum = ctx.enter_context(tc.tile_pool(name="cs", bufs=2, space="PSUM"))

    def emit_mm_group(gi, mlist=None, gmap=None, dcol=None):
        for m in (mlist if mlist is not None else range(8)):
            groups = gmap[m] if gmap else (
                R0_GROUPS[gi] if m < 4 else R1_GROUPS[gi])
            width = BLK * len(groups)
            ptf = mpsum.tile([128, 3 * BLK], F32, tag="mm",
                             name=f"pt{gi}_{m}_{len(groups)}")
            pt = ptf[:, :width]
            for i, t in enumerate(groups):
                nc.tensor.matmul(
                    pt[:, BLK * i:BLK * (i + 1)],
                    lhsT=zT[:, :, 128 * LIDX[m]:128 * (LIDX[m] + 1)],
                    rhs=zT[:, :, BLK * t:BLK * (t + 1)],
                    start=True, stop=True, perf_mode=DR,
                )
            slot = groups[0] if m < 4 else groups[0] - 8
            k = dcol[m] if dcol else gi * 8 + m
            eslot = esc[:, m, BLK * slot:BLK * slot + width]
            if (gi, m) in DVE_OFF and dcol is None:
                yi = ipool.tile([128, 3 * BLK], I32, tag="yi",
                                name=f"yi{gi}_{m}")
                nc.vector.tensor_scalar(
                    out=yi[:, :width], in0=pt[:], scalar1=SCH_A,
                    scalar2=SCH_B, op0=mult, op1=add,
                )
                nc.vector.scalar_tensor_tensor(
                    out=eslot, in0=yi[:, :width].bitcast(F32), scalar=1.0,
                    in1=onesw[:, :width], op0=mult, op1=mult,
                    accum_out=denacc[:, k:k + 1],
                )
            else:
                nc.scalar.activation(
                    eslot, pt[:], Exp, scale=INV_T,
                    accum_out=denacc[:, k:k + 1],
                )
        if dcol is None:
            nc.sync.dma_start(denacc_ap[:, gi * 8:gi * 8 + 8],
                              denacc[:, gi * 8:gi * 8 + 8])

    def emit_cs_group(gi):
        # column sums (DoubleRow: two m-tiles per matmul) of the
        # off-diagonal blocks whose exps were produced by group gi
        for bi, (mb, l) in enumerate(CS_BLOCKS):
            lset = R0_GROUPS[gi] if mb == 0 else R1_GROUPS[gi]
            if l not in lset or l == 0 or l == 8 and mb == 4:
                continue
            slot = l if mb == 0 else l - 8
            cs = cpsum.tile([1, BLK], F32, tag="cs", name=f"cs{bi}")
            for mm in range(4):
                nc.tensor.matmul(
                    cs[:], lhsT=ones[:, 0:1],
                    rhs=esc[:, mb + mm, BLK * slot:BLK * (slot + 1)],
                    start=(mm == 0), stop=(mm == 3),
                )
            nc.vector.tensor_copy(csb[0:1, BLK * bi:BLK * (bi + 1)], cs[:])
            nc.sync.dma_start(cs_ap[0:1, BLK * bi:BLK * (bi + 1)],
                              csb[0:1, BLK * bi:BLK * (bi + 1)])

    # ---- main loop: matmuls + fused exp/rowsum, colsums interleaved -----
    emit_mm_group(0)
    emit_mm_group(1)
    emit_cs_group(0)

    # positives: exact fp32 dots of normalized rows
    for m in range(4):
        junk = jpool.tile([128, D], F32, tag="junk", name=f"pp{m}")
        nc.vector.scalar_tensor_tensor(
            out=junk[:], in0=x[:, m, :], scalar=1.0,
            in1=x[:, 4 + m, :], op0=mult, op1=mult,
            accum_out=pos[:, m:m + 1],
        )

    emit_mm_group(2)
    emit_cs_group(1)
    emit_cs_group(2)

    nc.sync.dma_start(pos_ap[:], pos[:])


_CACHE = {}


def _get_compiled():
    if "nc" not in _CACHE:
        nc = bacc.Bacc("TRN2", target_bir_lowering=False, debug=False)
        zt_in = nc.dram_tensor("zt8", [128, 2, R], FP8, kind="ExternalInput")
        zrow_in = nc.dram_tensor("zrow", [1024, D], F32, kind="ExternalInput")
        den_out = nc.dram_tensor("denacc", [128, 26], F32, kind="ExternalOutput")
        cs_out = nc.dram_tensor("colsum", [1, NCS * BLK], F32, kind="ExternalOutput")
        pos_out = nc.dram_tensor("pos", [128, 4], F32, kind="ExternalOutput")
        with tile.TileContext(nc) as tc:
            _loss_kernel(tc, den_out.ap(), cs_out.ap(), pos_out.ap(),
                         zt_in.ap(), zrow_in.ap())
        nc.compile()
        _CACHE["nc"] = nc
    return _CACHE["nc"]


def make_in_maps(emb_i: np.ndarray, emb_j: np.ndarray):
    reps = np.concatenate(
        [np.asarray(emb_i, dtype=np.float32), np.asarray(emb_j, dtype=np.float32)],
        axis=0,
    )
    n = np.sqrt(np.sum(reps.astype(np.float64) ** 2, axis=1, keepdims=True))
    z = (reps / n).astype(np.float32)
    in_maps = []
    for c in range(N_CORES):
        zr = np.roll(z, -c * BLK, axis=0)
        # [128, 2, 8192] fp8: zt[p, k, col] = z[col, 128k + p]
        zt8 = np.ascontiguousarray(
            zr.reshape(R, 2, 128).transpose(2, 1, 0)
        ).astype(ml_dtypes.float8_e4m3)
        zrow = np.ascontiguousarray(
            np.concatenate([zr[0:BLK], zr[8 * BLK:9 * BLK]], axis=0))
        in_maps.append({"zt8": zt8, "zrow": zrow})
    return in_maps


def run_spmd(emb_i, emb_j, **kwargs):
    nc = _get_compiled()
    in_maps = make_in_maps(emb_i, emb_j)
    return run_bass_kernel_spmd(nc, in_maps, core_ids=list(range(N_CORES)), **kwargs)


def assemble(results) -> np.ndarray:
    denom = np.zeros(R, dtype=np.float64)
    pos2 = np.zeros(R, dtype=np.float64)
    for c in range(N_CORES):
        r = results[c]
        da = r["denacc"].astype(np.float64)        # [128, 24]
        cs = r["colsum"].reshape(NCS, BLK).astype(np.float64)
        pr = r["pos"].astype(np.float64)           # [128, 4]
        for m in range(8):
            s = da[:, m] + da[:, 8 + m] + da[:, 16 + m]
            if m == 0:
                s = s + da[:, 24]
            elif m == 4:
                s = s + da[:, 25]
            blk = c if m < 4 else c + 8
            g0 = blk * BLK + (m % 4) * 128
            denom[g0:g0 + 128] += s
        for bi, (mb, l) in enumerate(CS_BLOCKS):
            j = (c + l) % NBLK
            denom[j * BLK:(j + 1) * BLK] += cs[bi]
        for m in range(4):
            p2 = 2.0 * pr[:, m]
            g = BLK * c + m * 128 + np.arange(128)
            pos2[g] = p2
            pos2[(g + B) % R] = p2
    denom -= E2
    loss = float(np.mean(np.log(denom) - pos2))
    return np.array(loss, dtype=np.float32)


def kernel(emb_i: np.ndarray, emb_j: np.ndarray) -> np.ndarray:
    res = run_spmd(emb_i, emb_j)
    return assemble(res.results)


# revision 25
# speedup vs baseline: 1.1549x; 1.0157x over previous
"""NT-Xent / SimCLR contrastive loss on 8 Trainium2 NeuronCores.

Math (matches the jax reference):
    z = l2_normalize(concat([emb_i, emb_j]))          # [2B, D] unit rows
    sim = z @ z.T                                     # cosine similarities
    denom_r = sum_{j != r} exp(sim_rj / T)
    pos_r   = z_r . z_{(r+B) mod 2B}                  # the positive pair
    loss = mean_r( log(denom_r) - pos_r / T )

v10 — symmetric sharding + fp8 DoubleRow matmuls, host staging:
  sim is symmetric, so only the ~33.5M unique entries are exp'd (the
  exp on the ACT engine is the hard bottleneck: 1 elem/lane/cycle).
  The 8192 rows form 16 blocks of 512; core c owns row-blocks c and
  c+8 and computes blocks (c, c+l mod 16) for l=0..8 and
  (c+8, c+8+l mod 16) for l=0..7.  Every unordered block pair is
  covered exactly once (offsets 1..7 from each row-block, the 8 wrap
  pairs {c, c+8} internal to core c, 16 diagonals) — 17 blocks per
  core, balanced and SPMD-uniform.

  The host prepares the sharded operand layout: normalized rows,
  pre-transposed [d, row] panels, fp8e4 cast (bit-identical to the
  device DVE cast, verified RTN).  The device does the compute:
    - 68 fp8 DoubleRow matmuls (K=256 in one pass, 157 TF/s)
    - ACT exp(2*sim) straight out of PSUM, fused row accumulation,
      bf16 exp values to SBUF
    - 60 ones-matmuls on the PE for the 15 off-diagonal column sums
      (the transpose-side contributions), interleaved with the main
      loop so they ride in the ACT stream's shadow
    - five exp groups computed on the otherwise-idle DVE with a
      bias-tuned Schraudolph bit-trick exp, shaving the ACT stream
    - positives as exact fp32 row dots of the normalized rows
  Host assembles denom[8192] from row/col partials, subtracts the e^2
  self term, and takes mean(log(denom) - 2*pos) in float64.
"""

import numpy as np
from contextlib import ExitStack

import ml_dtypes
import concourse.bass as bass
import concourse.tile as tile
from concourse import bacc, mybir
from concourse._compat import with_exitstack
from concourse.bass_utils import run_bass_kernel_spmd

B = 4096
D = 256
R = 2 * B
N_CORES = 8
NBLK = 16            # global 512-row blocks
BLK = 512
INV_T = 2.0
E2 = float(np.exp(2.0))

F32 = mybir.dt.float32
BF16 = mybir.dt.bfloat16
FP8 = mybir.dt.float8e4
I32 = mybir.dt.int32
DR = mybir.MatmulPerfMode.DoubleRow
# Schraudolph exp on the DVE for a few groups (offloads the ACT
# bottleneck): exp(2*s) ~= bitcast_f32(int32(s*SCH_A + SCH_B)).
# SCH_B tuned on-device so the mean approximation ratio is 1.0.
SCH_A = 2.0 * (1 << 23) / np.log(2.0)
SCH_B = float(127 * (1 << 23) - 322200)
DVE_OFF = {(0, 5), (1, 2), (1, 6), (2, 1), (2, 5)}

# local col-block lists per exp group; row-block c tiles (m 0..3) use local
# cols 0..8 (diag, +1..+7, wrap), row-block c+8 tiles (m 4..7) use 8..15.
R0_GROUPS = [[0, 1, 2], [3, 4, 5], [6, 7, 8]]
R1_GROUPS = [[8, 9, 10], [11, 12, 13], [14, 15]]
# off-diagonal (row-tile-base, local col block) needing column sums
CS_BLOCKS = [(0, l) for l in range(1, 9)] + [(4, l) for l in range(9, 16)]
NCS = len(CS_BLOCKS)  # 15

# DMA block order: interleave the two half-panels so G0 unblocks first
B_SEQ = [b for g in range(8) for b in (g, g + 8)]
# local row-tile index backing matmul row m (tiles 0..3 = block c rows,
# zT cols 4096.. hold block c+8 = local block 8)
LIDX = [0, 1, 2, 3, 32, 33, 34, 35]


@with_exitstack
def _loss_kernel(ctx: ExitStack, tc: "tile.TileContext", denacc_ap: bass.AP,
                 cs_ap: bass.AP, pos_ap: bass.AP, zt_ap: bass.AP,
                 zrow_ap: bass.AP):
    nc = tc.nc
    mult = mybir.AluOpType.mult
    add = mybir.AluOpType.add
    Exp = mybir.ActivationFunctionType.Exp

    xpool = ctx.enter_context(tc.tile_pool(name="x", bufs=1))
    ipool = ctx.enter_context(tc.tile_pool(name="i32", bufs=2))
    jpool = ctx.enter_context(tc.tile_pool(name="junk", bufs=2))
    ztpool = ctx.enter_context(tc.tile_pool(name="zt", bufs=1))
    epool = ctx.enter_context(tc.tile_pool(name="esc", bufs=1))
    cpool = ctx.enter_context(tc.tile_pool(name="const", bufs=1))
    opool = ctx.enter_context(tc.tile_pool(name="outs", bufs=1))

    mpsum = ctx.enter_context(tc.tile_pool(name="mm", bufs=2, space="PSUM"))

    ones = cpool.tile([128, 1], BF16, tag="ones")
    nc.vector.memset(ones[:], 1.0)
    onesw = cpool.tile([128, 3 * BLK], F32, tag="onesw")
    nc.vector.memset(onesw[:], 1.0)

    zT = ztpool.tile([128, 2, R], FP8, tag="zt")
    # The gpsimd SWDGE issue costs ~770ns per dma_start, serialized; put
    # the six blocks the first matmul groups need on the idle HWDGE
    # queues (in need order), and fetch the rest as two span DMAs.
    nc.sync.dma_start(zT[:, :, 0:BLK * 3], zt_ap[:, :, 0:BLK * 3])
    nc.scalar.dma_start(zT[:, :, BLK * 8:BLK * 11],
                        zt_ap[:, :, BLK * 8:BLK * 11])
    for b in (3, 11, 4, 12, 5, 13, 6, 14, 7, 15):
        nc.gpsimd.dma_start(zT[:, :, BLK * b:BLK * (b + 1)],
                            zt_ap[:, :, BLK * b:BLK * (b + 1)])
    x = xpool.tile([128, 8, D], F32, tag="x")
    nc.sync.dma_start(
        x[:], zrow_ap[:].rearrange("(t p) d -> p t d", p=128))

    esc = epool.tile([128, 8, 9 * BLK], BF16, tag="esc")
    denacc = opool.tile([128, 26], F32, tag="denacc")
    pos = opool.tile([128, 4], F32, tag="pos")
    csb = opool.tile([1, NCS * BLK], F32, tag="csb")

    cpsum = ctx.enter_context(tc.tile_pool(name="cs", bufs=2, space="PSUM"))

    def emit_mm_group(gi, mlist=None, gmap=None, dcol=None):
        for m in (mlist if mlist is not None else range(8)):
            groups = gmap[m] if gmap else (
                R0_GROUPS[gi] if m < 4 else R1_GROUPS[gi])
            width = BLK * len(groups)
            ptf = mpsum.tile([128, 3 * BLK], F32, tag="mm",
                             name=f"pt{gi}_{m}_{len(groups)}")
            pt = ptf[:, :width]
            for i, t in enumerate(groups):
                nc.tensor.matmul(
                    pt[:, BLK * i:BLK * (i + 1)],
                    lhsT=zT[:, :, 128 * LIDX[m]:128 * (LIDX[m] + 1)],
                    rhs=zT[:, :, BLK * t:BLK * (t + 1)],
                    start=True, stop=True, perf_mode=DR,
                )
            slot = groups[0] if m < 4 else groups[0] - 8
            k = dcol[m] if dcol else gi * 8 + m
            eslot = esc[:, m, BLK * slot:BLK * slot + width]
            if (gi, m) in DVE_OFF and dcol is None:
                yi = ipool.tile([128, 3 * BLK], I32, tag="yi",
                                name=f"yi{gi}_{m}")
                nc.vector.tensor_scalar(
                    out=yi[:, :width], in0=pt[:], scalar1=SCH_A,
                    scalar2=SCH_B, op0=mult, op1=add,
                )
                nc.vector.scalar_tensor_tensor(
                    out=eslot, in0=yi[:, :width].bitcast(F32), scalar=1.0,
                    in1=onesw[:, :width], op0=mult, op1=mult,
                    accum_out=denacc[:, k:k + 1],
                )
            else:
                nc.scalar.activation(
                    eslot, pt[:], Exp, scale=INV_T,
                    accum_out=denacc[:, k:k + 1],
                )
        if dcol is None:
            nc.sync.dma_start(denacc_ap[:, gi * 8:gi * 8 + 8],
                              denacc[:, gi * 8:gi * 8 + 8])

    def emit_cs_group(gi):
        # column sums (DoubleRow: two m-tiles per matmul) of the
        # off-diagonal blocks whose exps were produced by group gi
        for bi, (mb, l) in enumerate(CS_BLOCKS):
            lset = R0_GROUPS[gi] if mb == 0 else R1_GROUPS[gi]
            if l not in lset or l == 0 or l == 8 and mb == 4:
                continue
            slot = l if mb == 0 else l - 8
            cs = cpsum.tile([1, BLK], F32, tag="cs", name=f"cs{bi}")
            for mm in range(4):
                nc.tensor.matmul(
                    cs[:], lhsT=ones[:, 0:1],
                    rhs=esc[:, mb + mm, BLK * slot:BLK * (slot + 1)],
                    start=(mm == 0), stop=(mm == 3),
                )
            nc.vector.tensor_copy(csb[0:1, BLK * bi:BLK * (bi + 1)], cs[:])
            nc.sync.dma_start(cs_ap[0:1, BLK * bi:BLK * (bi + 1)],
                              csb[0:1, BLK * bi:BLK * (bi + 1)])

    # ---- main loop: matmuls + fused exp/rowsum, colsums interleaved -----
    emit_mm_group(0)
    emit_mm_group(1)
    emit_cs_group(0)

    # positives: exact fp32 dots of normalized rows
    for m in range(4):
        junk = jpool.tile([128, D], F32, tag="junk", name=f"pp{m}")
        nc.vector.scalar_tensor_tensor(
            out=junk[:], in0=x[:, m, :], scalar=1.0,
            in1=x[:, 4 + m, :], op0=mult, op1=mult,
            accum_out=pos[:, m:m + 1],
        )

    emit_mm_group(2)
    emit_cs_group(1)
    emit_cs_group(2)

    nc.sync.dma_start(pos_ap[:], pos[:])


_CACHE = {}


def _get_compiled():
    if "nc" not in _CACHE:
        nc = bacc.Bacc("TRN2", target_bir_lowering=False, debug=False)
        zt_in = nc.dram_tensor("zt8", [128, 2, R], FP8, kind="ExternalInput")
        zrow_in = nc.dram_tensor("zrow", [1024, D], F32, kind="ExternalInput")
        den_out = nc.dram_tensor("denacc", [128, 26], F32, kind="ExternalOutput")
        cs_out = nc.dram_tensor("colsum", [1, NCS * BLK], F32, kind="ExternalOutput")
        pos_out = nc.dram_tensor("pos", [128, 4], F32, kind="ExternalOutput")
        with tile.TileContext(nc) as tc:
            _loss_kernel(tc, den_out.ap(), cs_out.ap(), pos_out.ap(),
                         zt_in.ap(), zrow_in.ap())
        nc.compile()
        _CACHE["nc"] = nc
    return _CACHE["nc"]


def make_in_maps(emb_i: np.ndarray, emb_j: np.ndarray):
    reps = np.concatenate(
        [np.asarray(emb_i, dtype=np.float32), np.asarray(emb_j, dtype=np.float32)],
        axis=0,
    )
    n = np.sqrt(np.sum(reps.astype(np.float64) ** 2, axis=1, keepdims=True))
    z = (reps / n).astype(np.float32)
    in_maps = []
    for c in range(N_CORES):
        zr = np.roll(z, -c * BLK, axis=0)
        # [128, 2, 8192] fp8: zt[p, k, col] = z[col, 128k + p]
        zt8 = np.ascontiguousarray(
            zr.reshape(R, 2, 128).transpose(2, 1, 0)
        ).astype(ml_dtypes.float8_e4m3)
        zrow = np.ascontiguousarray(
            np.concatenate([zr[0:BLK], zr[8 * BLK:9 * BLK]], axis=0))
        in_maps.append({"zt8": zt8, "zrow": zrow})
    return in_maps


def run_spmd(emb_i, emb_j, **kwargs):
    nc = _get_compiled()
    in_maps = make_in_maps(emb_i, emb_j)
    return run_bass_kernel_spmd(nc, in_maps, core_ids=list(range(N_CORES)), **kwargs)


def assemble(results) -> np.ndarray:
    denom = np.zeros(R, dtype=np.float64)
    pos2 = np.zeros(R, dtype=np.float64)
    for c in range(N_CORES):
        r = results[c]
        da = r["denacc"].astype(np.float64)        # [128, 24]
        cs = r["colsum"].reshape(NCS, BLK).astype(np.float64)
        pr = r["pos"].astype(np.float64)           # [128, 4]
        for m in range(8):
            s = da[:, m] + da[:, 8 + m] + da[:, 16 + m]
            if m == 0:
                s = s + da[:, 24]
            elif m == 4:
                s = s + da[:, 25]
            blk = c if m < 4 else c + 8
            g0 = blk * BLK + (m % 4) * 128
            denom[g0:g0 + 128] += s
        for bi, (mb, l) in enumerate(CS_BLOCKS):
            j = (c + l) % NBLK
            denom[j * BLK:(j + 1) * BLK] += cs[bi]
        for m in range(4):
            p2 = 2.0 * pr[:, m]
            g = BLK * c + m * 128 + np.arange(128)
            pos2[g] = p2
            pos2[(g + B) % R] = p2
    denom -= E2
    loss = float(np.mean(np.log(denom) - pos2))
    return np.array(loss, dtype=np.float32)


def kernel(emb_i: np.ndarray, emb_j: np.ndarray) -> np.ndarray:
    res = run_spmd(emb_i, emb_j)
    return assemble(res.results)
